# revision 4
# baseline (speedup 1.0000x reference)
"""MinLSTM Trainium2 kernel (v3).

Problem: B=8, S=4096, In=512, H=512 (fp32).
    f_t = sigmoid(x @ W_f^T + b_f); i_t = sigmoid(x @ W_i^T + b_i)
    h_tilde = x @ W_h^T + b_h
    f_n = f_t / (f_t + i_t + eps); i_n = i_t / (f_t + i_t + eps)
    h_t = f_n * h_{t-1} + i_n * h_tilde   (scan over S)

Strategy: data-parallel over batch — 1 sample per NeuronCore (8 cores).
Per-core layout is transposed: [H on partitions, S on free dim].

v3 changes vs v2 (which ran ~94.5us/iter):
  - MINLSTM_PAIR_SCAN: a hand-written DVE uop program for the affine
    recurrence h_t = f*h_{t-1} + g at 1 elem/cycle (stock
    tensor_tensor_scan runs 2 cyc/elem: its mult+add feedback spans two
    ALU stages, forcing a bubble uop every element). The 2X_1PORT
    program consumes bf16 PAIRS (rd0: f_{2k},f_{2k+1}; rd1: g pair) per
    work/bubble round: locals c=f1*f0, d=f1*g0+g1 (blocks 0-2), the
    2-stage feedback round m=c*A, h2=m+d (blocks 3-4, A<-h2), then
    h1=f0*A_prev+g0 (blocks 5-6), packing (h1,h2) into one 32-bit write.
    Same work/bubble FSM as stock TTSA, but 2 elements per round.
  - One full-row scan per h-block ([128, 4096]) instead of two 2048
    chunks: amortizes instruction overhead, and the init stays an fp32
    AP (h_0) — the imm path requires fp32.
  - hh and the output DMA in bf16 (halves store traffic).
  - repeat loop unrolled 8x inside For_i; the per-iteration pending-g
    chain flows between unrolled bodies. The loop edge otherwise costs
    ~16us/iter in lost overlap (tail scan+DMA serialize against the
    next trip).
Measured ~67-70us/iter (quiet machine; ambient bursts can inflate
samples) vs 94.5 for v2. The fp8/bf16 matmul stream alone is 131072
PE cycles and measures 64.5-68us at the chip's sustained ~1.98-2.03
GHz clock (P0 power state), so the kernel is tensor-engine-bound at
the silicon's clock: 512- vs 256-wide MM streams time identically
(zero per-MM overhead, LDWEIGHTS fully hidden), and DoubleRow shows
no per-MM penalty here. Cheaper h_tilde paths (fp8 / partial-fp8)
are barred: fp8 quantization of x and W each pass ~2.1% rms straight
to the output (the scan's AR(1) filter attenuates signal and noise
equally), and DoubleRow's K=256 granularity makes a quarter-fp8
split save zero cycles.
"""

from dataclasses import dataclass, field

import numpy as np
import ml_dtypes

import concourse.bass as bass
import concourse.bacc as bacc
import concourse.tile as tile
from concourse import mybir
from concourse.bass import ts, ds
from concourse.bass_utils import run_bass_kernel_spmd

BF16 = ml_dtypes.bfloat16

B, S, IN, H = 8, 4096, 512, 512
KI = IN // 128        # 4 k-tiles of the contraction dim
KI2 = IN // 256       # 2 DoubleRow k-tiles (fp8 path)
HB = H // 128         # 4 h blocks (partition blocks)
CH = 2048             # S-chunk per PSUM tile (4 banks)
NCH = S // CH         # 2 chunks
MM = 512              # matmul free dim (1 PSUM bank)
UNROLL = 8            # bodies per For_i trip in repeat mode

USE_FP8 = True        # fp8e4m3 DoubleRow for the f/i gate matmuls
FP8_SC = 64.0         # weight pre-scale (undone via ACT scale=1/FP8_SC)
FP8 = ml_dtypes.float8_e4m3

# Chebyshev-minimax constants for the bitwise-not reciprocal seed
# (same interval as RECIP_APPROX_FAST_CONSTS in concourse.dve_ops).
_RC0 = -0.23549792
_RC1 = 2.0017324

_CACHE = {}


def _register_gate_fn():
    """Register the fused gate op: out = in0 * approx(1/(in0+in1)).

    s = f+i; seed y0 = bitcast(~s)*C0; y1 = y0*(C1 - s*y0); out = f*y1.
    7 ALU stages, one DVE instruction at 1 elem/cycle/lane (fp32 streams).
    Max rel err of fn vs exact f/(f+i): ~1.7e-3.
    """
    import concourse.dve_ops as D

    for op in D.OPS:
        if op.name == "MINLSTM_GATE_FN":
            return op

    from concourse.dve_spec import Spec, Src0, Src1, C0, C1, Bin, AluOp, lower
    from concourse.dve_uop import DveOpSpec

    _s = Src0 + Src1
    _ns = Bin(AluOp.BITWISE_NOT, _s, _s)
    _y0 = _ns * C0
    _y1 = _y0 * (C1 - _s * _y0)

    def _ref(in0, in1, s0, s1, imm2):
        s = (in0 + in1).astype(np.float32)
        not_s = (~s.view(np.int32)).view(np.float32)
        y0 = not_s * np.float32(s0)
        y1 = y0 * (np.float32(s1) - s * y0)
        return in0 * y1

    spec = Spec(body=Src0 * _y1, reference=_ref)
    shas = {}
    op = D.DveOp("MINLSTM_GATE_FN", spec, subdim=False, uops_sha=shas)
    D.OPS.append(op)
    D.CUSTOM_DVE_SPECS[op.name] = spec
    D._SUB_OPCODE_FOR_NAME[op.name] = D._CUSTOM_DVE_ROW_BASE + len(D.OPS) - 1
    opcode = D.get_dve_sub_opcode(op.name)
    for ver in ("v3", "v4"):
        s = DveOpSpec(
            name=op.name, opcode=opcode, uops=lower(spec, ver=ver), rd1_en=True
        )
        shas[ver] = s.sha(ver)
    return op


# --------------------------------------------------------------------------
# MINLSTM_PAIR_SCAN — hand-written DVE uop program for the affine scan
# h_t = f_t * h_{t-1} + g_t at 1 elem/cycle (2X_1PORT bf16-pair program;
# REGULAR slot carries a stock-TTSA-equivalent 1x fallback).
# --------------------------------------------------------------------------

_PAIR_SCAN_NAME = "MINLSTM_PAIR_SCAN"


def _pair_scan_ref(in0, in1, s0, s1, imm2):
    f = np.asarray(in0, np.float32)
    g = np.asarray(in1, np.float32)
    h = np.empty_like(f)
    if np.isscalar(s0) or np.ndim(s0) == 0:
        prev = np.full((f.shape[0],), np.float32(s0))
    else:
        prev = np.asarray(s0, np.float32).reshape(f.shape[0])
    for t in range(f.shape[1]):
        prev = f[:, t] * prev + g[:, t]
        h[:, t] = prev
    return h


def _register_pair_scan():
    import concourse.dve_ops as D
    from concourse.dve_spec import Spec, Src0, Src1, C0
    from concourse.dve_uop import (
        AluInp, AluOp, DelayInp, DveOpSpec, ENABLE, InpSel, OutPath, OutSel,
        Trigger, UopConfig,
    )

    for op in D.OPS:
        if op.name == _PAIR_SCAN_NAME:
            return op

    @dataclass(frozen=True)
    class _RawDveOp(D.DveOp):
        raw: dict = field(default_factory=dict)

        def compile(self, ver):
            return self.raw[ver]

    def _seed(a_block):
        # Push C0 (the per-partition init) down the ALU chain; load the A
        # (feedback) flop at a_block.
        u = UopConfig()
        u.enable_input(InpSel.CONST_0, 0)
        u.repeat_count = 1
        u.trigger = (Trigger.COUNT, Trigger.NONE, Trigger.NONE)
        u.next_uop = (1, 0, 0)
        for b in range(a_block + 1):
            u.datapath_config[b].pass_through_alu()
        u.datapath_config[a_block].alu_out_a_enable = ENABLE
        return u

    def _bubble():
        u = UopConfig()
        u.repeat_count = 1
        u.trigger = (Trigger.COUNT, Trigger.NONE, Trigger.NONE)
        u.next_uop = (2, 0, 0)
        return u

    def _work_regular():
        # 1x: m = f*A (blk0); h = m + g (blk1, A <- h). f=inp0, g=lane0.
        u = UopConfig()
        u.enable_input(InpSel.SRC_0, 0)
        u.enable_input(InpSel.SRC_1, 1)
        u.repeat_count = 1
        u.require_inp0 = ENABLE
        u.require_inp1 = ENABLE
        u.trigger = (Trigger.SRC_TENSOR_DONE, Trigger.COUNT, Trigger.NONE)
        u.next_uop = (0, 1, 0)
        dp = u.datapath_config
        dp[0].enable_alu(
            AluOp.MULTIPLY, AluInp.PREV_ALU_OUT, AluInp.NEXT_ALU_OUT_A)
        dp[0].pass_through_delay(0)
        dp[1].enable_alu(AluOp.ADD, AluInp.PREV_ALU_OUT, AluInp.PREV_DELAY_0)
        dp[1].alu_out_a_enable = ENABLE
        for b in range(2, 8):
            dp[b].pass_through_alu()
        u.enable_output(OutSel.ALU_OUT, OutPath.WR0_LO)
        return u

    def _work_pair():
        # 2X_1PORT: lanes 0<-f1(SRC_0_HI), 1<-g0(SRC_1), 2<-g1(SRC_1_HI),
        # 3<-f0(SRC_0); inp0 = f0 feeds blk0's ALU directly.
        u = UopConfig()
        u.enable_input(InpSel.SRC_0, 0)
        u.enable_input(InpSel.SRC_0_HI, 1)
        u.enable_input(InpSel.SRC_1, 2)
        u.enable_input(InpSel.SRC_1_HI, 3)
        u.enable_input(InpSel.SRC_0, 4)
        u.repeat_count = 1
        u.require_inp0 = ENABLE
        u.require_inp1 = ENABLE
        u.trigger = (Trigger.SRC_TENSOR_DONE, Trigger.COUNT, Trigger.NONE)
        u.next_uop = (0, 1, 0)
        dp = u.datapath_config
        # blk0: c = f0 * f1
        dp[0].enable_alu(
            AluOp.MULTIPLY, AluInp.PREV_ALU_OUT, AluInp.PREV_DELAY_0)
        dp[0].pass_through_delay(0, 1, 2, 3)
        # blk1: t = f1 * g0 ; lane4 <- c
        dp[1].enable_alu(
            AluOp.MULTIPLY, AluInp.PREV_DELAY_0, AluInp.PREV_DELAY_1)
        dp[1].pass_through_delay(1, 2, 3)
        dp[1].enable_delay_from_src(DelayInp.PREV_ALU_OUT, 4)
        # blk2: d = t + g1
        dp[2].enable_alu(AluOp.ADD, AluInp.PREV_ALU_OUT, AluInp.PREV_DELAY_2)
        dp[2].pass_through_delay(1, 3, 4)
        # blk3: m = c * A ; lane5 <- d ; lane0 <- A (capture h_prev)
        dp[3].enable_alu(
            AluOp.MULTIPLY, AluInp.PREV_DELAY_4, AluInp.NEXT_ALU_OUT_A)
        dp[3].pass_through_delay(1, 3)
        dp[3].enable_delay_from_src(DelayInp.PREV_ALU_OUT, 5)
        dp[3].enable_delay_from_src(DelayInp.NEXT_ALU_OUT_A, 0)
        # blk4: h2 = m + d ; A <- h2
        dp[4].enable_alu(AluOp.ADD, AluInp.PREV_ALU_OUT, AluInp.PREV_DELAY_5)
        dp[4].alu_out_a_enable = ENABLE
        dp[4].pass_through_delay(0, 1, 3)
        # blk5: m1 = f0 * h_prev ; lane2 <- h2
        dp[5].enable_alu(
            AluOp.MULTIPLY, AluInp.PREV_DELAY_3, AluInp.PREV_DELAY_0)
        dp[5].pass_through_delay(1)
        dp[5].enable_delay_from_src(DelayInp.PREV_ALU_OUT, 2)
        # blk6: h1 = m1 + g0
        dp[6].enable_alu(AluOp.ADD, AluInp.PREV_ALU_OUT, AluInp.PREV_DELAY_1)
        dp[6].pass_through_delay(2)
        # blk7: pass h1 on the ALU; keep h2 on lane2
        dp[7].pass_through_alu()
        dp[7].pass_through_delay(2)
        u.enable_output(OutSel.ALU_OUT, OutPath.WR0_LO)   # h1 (elem 2k)
        u.enable_output(OutSel.DELAY_2, OutPath.WR0_HI)   # h2 (elem 2k+1)
        return u

    spec = Spec(body=Src0 * C0 + Src1, reference=_pair_scan_ref)
    op = _RawDveOp(_PAIR_SCAN_NAME, spec, subdim=False, uops_sha={})
    D.OPS.append(op)
    D.CUSTOM_DVE_SPECS[op.name] = spec
    D._SUB_OPCODE_FOR_NAME[op.name] = D._CUSTOM_DVE_ROW_BASE + len(D.OPS) - 1
    opcode = D.get_dve_sub_opcode(op.name)
    for ver in ("v3", "v4"):
        sp = DveOpSpec(
            name=op.name,
            opcode=opcode,
            uops=[_seed(1), _bubble(), _work_regular()],
            uops_2x=[_seed(4), _bubble(), _work_pair()],
            perf_max=1,
            rd1_en=True,
        )
        sp.validate(ver)
        op.raw[ver] = sp
    return op


def _emit_pair_scan(nc, op, out, in0, in1, s0):
    inst = nc.vector._custom_dve(op, out=out, in0=in0, in1=in1, s0=s0)
    inst.ins.perf_max = 1  # engage the 2X_1PORT slot (byte-36[7:6])
    return inst


def build_minlstm_bass(repeat=1, use_fp8=None):
    if use_fp8 is None:
        use_fp8 = USE_FP8
    assert use_fp8, "v3 kernel is fp8-gate only"
    gate_fn_op = _register_gate_fn()
    pair_scan_op = _register_pair_scan()

    nc = bacc.Bacc("TRN2", debug=False, num_devices=B)
    f32 = mybir.dt.float32
    bf16 = mybir.dt.bfloat16
    fp8 = mybir.dt.float8e4

    xT = nc.dram_tensor("xt", [KI, 128, S], bf16, kind="ExternalInput").ap()
    xT8 = nc.dram_tensor(
        "xt8", [KI2, 128, 2, S], fp8, kind="ExternalInput").ap()
    wf8T = nc.dram_tensor(
        "wf8t", [KI2, 128, 2, H], fp8, kind="ExternalInput").ap()
    wi8T = nc.dram_tensor(
        "wi8t", [KI2, 128, 2, H], fp8, kind="ExternalInput").ap()
    whnT = nc.dram_tensor("whnt", [KI, 128, H], bf16, kind="ExternalInput").ap()
    bfb = nc.dram_tensor("bfb", [128, HB], f32, kind="ExternalInput").ap()
    bib = nc.dram_tensor("bib", [128, HB], f32, kind="ExternalInput").ap()
    bhnb = nc.dram_tensor("bhnb", [128, HB], f32, kind="ExternalInput").ap()
    h0b = nc.dram_tensor("h0b", [128, HB], f32, kind="ExternalInput").ap()
    outT = nc.dram_tensor(
        "outt", [HB, 128, S], bf16, kind="ExternalOutput").ap()

    Sig = mybir.ActivationFunctionType.Sigmoid
    Ident = mybir.ActivationFunctionType.Identity
    Alu = mybir.AluOpType

    with tile.TileContext(nc) as tc, nc.allow_low_precision(reason="bf16 gates"):
        with (
            tc.tile_pool(name="const", bufs=1) as const,
            tc.tile_pool(name="ps", bufs=2, space="PSUM") as ps,
            tc.tile_pool(name="p_sf", bufs=2) as p_sf,
            tc.tile_pool(name="p_si", bufs=2) as p_si,
            tc.tile_pool(name="p_htn", bufs=3) as p_htn,
            tc.tile_pool(name="p_inn", bufs=3) as p_inn,
            tc.tile_pool(name="p_fn", bufs=2) as p_fn,
            tc.tile_pool(name="p_g", bufs=2) as p_g,
            tc.tile_pool(name="hout", bufs=2) as hout,
        ):
            whn_sb = const.tile([128, KI, H], bf16, tag="whn")
            x_sb = const.tile([128, KI, S], bf16, tag="x")
            wf8_sb = const.tile([128, KI2, 2, H], fp8, tag="wf8")
            wi8_sb = const.tile([128, KI2, 2, H], fp8, tag="wi8")
            x8_sb = const.tile([128, KI2, 2, S], fp8, tag="x8")
            for ki2 in range(KI2):
                nc.sync.dma_start(
                    out=wf8_sb[:, ki2, :, :], in_=wf8T[ki2, :, :, :])
                nc.sync.dma_start(
                    out=wi8_sb[:, ki2, :, :], in_=wi8T[ki2, :, :, :])
            for ki in range(KI):
                nc.sync.dma_start(out=whn_sb[:, ki, :], in_=whnT[ki, :, :])
            for ch in range(NCH):
                for ki2 in range(KI2):
                    nc.sync.dma_start(
                        out=x8_sb[:, ki2, :, ts(ch, CH)],
                        in_=xT8[ki2, :, :, ts(ch, CH)])
                for ki in range(KI):
                    nc.sync.dma_start(
                        out=x_sb[:, ki, ts(ch, CH)], in_=xT[ki, :, ts(ch, CH)])
            bf_sb = const.tile([128, HB], f32, tag="bf")
            bi_sb = const.tile([128, HB], f32, tag="bi")
            bhn_sb = const.tile([128, HB], f32, tag="bhn")
            h0_sb = const.tile([128, HB], f32, tag="h0")
            nc.sync.dma_start(out=bf_sb, in_=bfb[:, :])
            nc.sync.dma_start(out=bi_sb, in_=bib[:, :])
            nc.sync.dma_start(out=bhn_sb, in_=bhnb[:, :])
            nc.sync.dma_start(out=h0_sb, in_=h0b[:, :])

            def gate_matmul(w_sb, hb, ch):
                pp = ps.tile([128, CH], f32, tag="pp")
                for ki in range(KI):
                    st, sp = (ki == 0), (ki == KI - 1)
                    for c in range(CH // MM):
                        nc.tensor.matmul(
                            pp[:, ts(c, MM)],
                            w_sb[:, ki, ds(hb * 128, 128)],
                            x_sb[:, ki, ds(ch * CH + c * MM, MM)],
                            start=st, stop=sp)
                return pp

            def gate_matmul_fp8(w8_sb, hb, ch):
                pp = ps.tile([128, CH], f32, tag="pp")
                for ki2 in range(KI2):
                    st, sp = (ki2 == 0), (ki2 == KI2 - 1)
                    for c in range(CH // MM):
                        nc.tensor.matmul(
                            pp[:, ts(c, MM)],
                            w8_sb[:, ki2, :, ds(hb * 128, 128)],
                            x8_sb[:, ki2, :, ds(ch * CH + c * MM, MM)],
                            start=st, stop=sp,
                            perf_mode=mybir.MatmulPerfMode.DoubleRow)
                return pp

            def emit_g(p, last):
                """Emit pending chunk p's g; close the row (scan+DMA) at
                ch==NCH-1. The final chunk uses one STT so its tail skips
                the GPSIMD hop."""
                hb, ch, fn_row, g_row, hh, htn, inn = p
                gsl = g_row[:, ts(ch, CH)]
                if last:
                    nc.vector.scalar_tensor_tensor(
                        gsl, fn_row[:, ts(ch, CH)], 1.0, htn,
                        Alu.subtract, Alu.mult)
                else:
                    nc.vector.tensor_tensor(gsl, inn, htn, Alu.mult)
                if ch == NCH - 1:
                    _emit_pair_scan(
                        nc, pair_scan_op, hh, fn_row, g_row,
                        h0_sb[:, hb : hb + 1])
                    eng = nc.gpsimd if (hb % 2) else nc.sync
                    eng.dma_start(out=outT[hb, :, :], in_=hh)

            def body(_i=None, pend=None, close=True):
                for hb in range(HB):
                    fn_row = p_fn.tile([128, S], bf16, tag="fn")
                    g_row = p_g.tile([128, S], bf16, tag="g")
                    hh = hout.tile([128, S], bf16, tag="hh")
                    for ch in range(NCH):
                        sc = 1.0 / FP8_SC
                        if pend is not None:
                            emit_g(pend, last=False)
                            pend = None
                        ppf = gate_matmul_fp8(wf8_sb, hb, ch)
                        sf = p_sf.tile([128, CH], f32, tag="sf")
                        nc.scalar.activation(
                            sf, ppf, Sig, bias=bf_sb[:, hb : hb + 1], scale=sc)
                        ppi = gate_matmul_fp8(wi8_sb, hb, ch)
                        si = p_si.tile([128, CH], f32, tag="si")
                        nc.scalar.activation(
                            si, ppi, Sig, bias=bi_sb[:, hb : hb + 1], scale=sc)
                        pph = gate_matmul(whn_sb, hb, ch)
                        htn = p_htn.tile([128, CH], bf16, tag="htn")
                        nc.scalar.activation(
                            htn, pph, Ident, bias=bhn_sb[:, hb : hb + 1])

                        nc.vector._custom_dve(
                            gate_fn_op, out=fn_row[:, ts(ch, CH)],
                            in0=sf, in1=si, s0=_RC0, s1=_RC1)
                        inn = p_inn.tile([128, CH], bf16, tag="inn")
                        nc.gpsimd.tensor_scalar(
                            inn, fn_row[:, ts(ch, CH)], 1.0, -1.0,
                            Alu.mult, Alu.add)
                        pend = (hb, ch, fn_row, g_row, hh, htn, inn)
                if close:
                    emit_g(pend, last=True)
                    return None
                return pend

            if repeat == 1:
                body()
            else:
                n_loop = repeat // UNROLL
                if n_loop > 0:
                    with tc.For_i(0, n_loop, 1) as _i:
                        p = None
                        for _u in range(UNROLL):
                            p = body(_i, pend=p, close=(_u == UNROLL - 1))
                for _u in range(repeat - max(n_loop, 0) * UNROLL):
                    body()
    nc.compile()
    return nc


def _dr8(W):
    """[H, In] -> DoubleRow fp8 layout [KI2, 128, 2, H]: plane pl of ki2
    holds k-tile (2*ki2+pl)."""
    return np.ascontiguousarray(
        W.T.reshape(KI2, 2, 128, H).transpose(0, 2, 1, 3).astype(FP8))


def _prep_core_inputs(x, h_0, W_f, b_f, W_i, b_i, W_h, b_h):
    """Build per-core input maps (host-side shard + layout transform)."""
    shared = {}
    shared["wf8t"] = _dr8(W_f * FP8_SC)
    shared["wi8t"] = _dr8(W_i * FP8_SC)
    shared["whnt"] = np.ascontiguousarray(
        (-W_h).T.reshape(KI, 128, H).astype(BF16))
    shared["bfb"] = np.ascontiguousarray(
        b_f.reshape(HB, 128).T.astype(np.float32))
    shared["bib"] = np.ascontiguousarray(
        b_i.reshape(HB, 128).T.astype(np.float32))
    shared["bhnb"] = np.ascontiguousarray(
        (-b_h).reshape(HB, 128).T.astype(np.float32))
    in_maps = []
    for b in range(B):
        m = dict(shared)
        m["xt"] = np.ascontiguousarray(x[b].T.reshape(KI, 128, S).astype(BF16))
        m["xt8"] = np.ascontiguousarray(
            x[b].T.reshape(KI2, 2, 128, S).transpose(0, 2, 1, 3).astype(FP8))
        m["h0b"] = np.ascontiguousarray(
            h_0[b].reshape(HB, 128).T.astype(np.float32))
        in_maps.append(m)
    return in_maps


def _run(in_maps, trace=False, repeat=1):
    key = f"nc{repeat}"
    if key not in _CACHE:
        _CACHE[key] = build_minlstm_bass(repeat=repeat)
    return run_bass_kernel_spmd(
        _CACHE[key], in_maps, core_ids=list(range(B)), trace=trace)


def kernel(x, h_0, W_f, b_f, W_i, b_i, W_h, b_h):
    x = np.asarray(x, dtype=np.float32)
    h_0 = np.asarray(h_0, dtype=np.float32)
    in_maps = _prep_core_inputs(
        x, h_0,
        np.asarray(W_f, np.float32), np.asarray(b_f, np.float32),
        np.asarray(W_i, np.float32), np.asarray(b_i, np.float32),
        np.asarray(W_h, np.float32), np.asarray(b_h, np.float32))
    res = _run(in_maps)
    out = np.empty((B, S, H), dtype=np.float32)
    for b in range(B):
        outt = res.results[b]["outt"]  # [HB, 128, S] bf16
        out[b] = outt.reshape(H, S).T.astype(np.float32)
    return out


# revision 5
# speedup vs baseline: 1.0204x; 1.0204x over previous
"""MinLSTM Trainium2 kernel (v3).

Problem: B=8, S=4096, In=512, H=512 (fp32).
    f_t = sigmoid(x @ W_f^T + b_f); i_t = sigmoid(x @ W_i^T + b_i)
    h_tilde = x @ W_h^T + b_h
    f_n = f_t / (f_t + i_t + eps); i_n = i_t / (f_t + i_t + eps)
    h_t = f_n * h_{t-1} + i_n * h_tilde   (scan over S)

Strategy: data-parallel over batch — 1 sample per NeuronCore (8 cores).
Per-core layout is transposed: [H on partitions, S on free dim].

v3 changes vs v2 (which ran ~94.5us/iter):
  - MINLSTM_PAIR_SCAN: a hand-written DVE uop program for the affine
    recurrence h_t = f*h_{t-1} + g at 1 elem/cycle (stock
    tensor_tensor_scan runs 2 cyc/elem: its mult+add feedback spans two
    ALU stages, forcing a bubble uop every element). The 2X_1PORT
    program consumes bf16 PAIRS (rd0: f_{2k},f_{2k+1}; rd1: g pair) per
    work/bubble round: locals c=f1*f0, d=f1*g0+g1 (blocks 0-2), the
    2-stage feedback round m=c*A, h2=m+d (blocks 3-4, A<-h2), then
    h1=f0*A_prev+g0 (blocks 5-6), packing (h1,h2) into one 32-bit write.
    Same work/bubble FSM as stock TTSA, but 2 elements per round.
  - One full-row scan per h-block ([128, 4096]) instead of two 2048
    chunks: amortizes instruction overhead, and the init stays an fp32
    AP (h_0) — the imm path requires fp32.
  - hh and the output DMA in bf16 (halves store traffic).
  - repeat loop unrolled 8x inside For_i; the per-iteration pending-g
    chain flows between unrolled bodies. The loop edge otherwise costs
    ~16us/iter in lost overlap (tail scan+DMA serialize against the
    next trip).
Measured ~67-70us/iter (quiet machine; ambient bursts can inflate
samples) vs 94.5 for v2. The fp8/bf16 matmul stream alone is 131072
PE cycles and measures 64.5-68us at the chip's sustained ~1.98-2.03
GHz clock (P0 power state), so the kernel is tensor-engine-bound at
the silicon's clock: 512- vs 256-wide MM streams time identically
(zero per-MM overhead, LDWEIGHTS fully hidden), and DoubleRow shows
no per-MM penalty here. Cheaper h_tilde paths (fp8 / partial-fp8)
are barred: fp8 quantization of x and W each pass ~2.1% rms straight
to the output (the scan's AR(1) filter attenuates signal and noise
equally), and DoubleRow's K=256 granularity makes a quarter-fp8
split save zero cycles.
"""

from dataclasses import dataclass, field

import numpy as np
import ml_dtypes

import concourse.bass as bass
import concourse.bacc as bacc
import concourse.tile as tile
from concourse import mybir
from concourse.bass import ts, ds
from concourse.bass_utils import run_bass_kernel_spmd

BF16 = ml_dtypes.bfloat16

B, S, IN, H = 8, 4096, 512, 512
KI = IN // 128        # 4 k-tiles of the contraction dim
KI2 = IN // 256       # 2 DoubleRow k-tiles (fp8 path)
HB = H // 128         # 4 h blocks (partition blocks)
CH = 2048             # S-chunk per PSUM tile (4 banks)
NCH = S // CH         # 2 chunks
MM = 512              # matmul free dim (1 PSUM bank)
UNROLL = 16           # bodies per For_i trip in repeat mode

USE_FP8 = True        # fp8e4m3 DoubleRow for the f/i gate matmuls
FP8_SC = 64.0         # weight pre-scale (undone via ACT scale=1/FP8_SC)
FP8 = ml_dtypes.float8_e4m3

# Chebyshev-minimax constants for the bitwise-not reciprocal seed
# (same interval as RECIP_APPROX_FAST_CONSTS in concourse.dve_ops).
_RC0 = -0.23549792
_RC1 = 2.0017324

_CACHE = {}


def _register_gate_fn():
    """Register the fused gate op: out = in0 * approx(1/(in0+in1)).

    s = f+i; seed y0 = bitcast(~s)*C0; y1 = y0*(C1 - s*y0); out = f*y1.
    7 ALU stages, one DVE instruction at 1 elem/cycle/lane (fp32 streams).
    Max rel err of fn vs exact f/(f+i): ~1.7e-3.
    """
    import concourse.dve_ops as D

    for op in D.OPS:
        if op.name == "MINLSTM_GATE_FN":
            return op

    from concourse.dve_spec import Spec, Src0, Src1, C0, C1, Bin, AluOp, lower
    from concourse.dve_uop import DveOpSpec

    _s = Src0 + Src1
    _ns = Bin(AluOp.BITWISE_NOT, _s, _s)
    _y0 = _ns * C0
    _y1 = _y0 * (C1 - _s * _y0)

    def _ref(in0, in1, s0, s1, imm2):
        s = (in0 + in1).astype(np.float32)
        not_s = (~s.view(np.int32)).view(np.float32)
        y0 = not_s * np.float32(s0)
        y1 = y0 * (np.float32(s1) - s * y0)
        return in0 * y1

    spec = Spec(body=Src0 * _y1, reference=_ref)
    shas = {}
    op = D.DveOp("MINLSTM_GATE_FN", spec, subdim=False, uops_sha=shas)
    D.OPS.append(op)
    D.CUSTOM_DVE_SPECS[op.name] = spec
    D._SUB_OPCODE_FOR_NAME[op.name] = D._CUSTOM_DVE_ROW_BASE + len(D.OPS) - 1
    opcode = D.get_dve_sub_opcode(op.name)
    for ver in ("v3", "v4"):
        s = DveOpSpec(
            name=op.name, opcode=opcode, uops=lower(spec, ver=ver), rd1_en=True
        )
        shas[ver] = s.sha(ver)
    return op


# --------------------------------------------------------------------------
# MINLSTM_PAIR_SCAN — hand-written DVE uop program for the affine scan
# h_t = f_t * h_{t-1} + g_t at 1 elem/cycle (2X_1PORT bf16-pair program;
# REGULAR slot carries a stock-TTSA-equivalent 1x fallback).
# --------------------------------------------------------------------------

_PAIR_SCAN_NAME = "MINLSTM_PAIR_SCAN"


def _pair_scan_ref(in0, in1, s0, s1, imm2):
    f = np.asarray(in0, np.float32)
    g = np.asarray(in1, np.float32)
    h = np.empty_like(f)
    if np.isscalar(s0) or np.ndim(s0) == 0:
        prev = np.full((f.shape[0],), np.float32(s0))
    else:
        prev = np.asarray(s0, np.float32).reshape(f.shape[0])
    for t in range(f.shape[1]):
        prev = f[:, t] * prev + g[:, t]
        h[:, t] = prev
    return h


def _register_pair_scan():
    import concourse.dve_ops as D
    from concourse.dve_spec import Spec, Src0, Src1, C0
    from concourse.dve_uop import (
        AluInp, AluOp, DelayInp, DveOpSpec, ENABLE, InpSel, OutPath, OutSel,
        Trigger, UopConfig,
    )

    for op in D.OPS:
        if op.name == _PAIR_SCAN_NAME:
            return op

    @dataclass(frozen=True)
    class _RawDveOp(D.DveOp):
        raw: dict = field(default_factory=dict)

        def compile(self, ver):
            return self.raw[ver]

    def _seed(a_block):
        # Push C0 (the per-partition init) down the ALU chain; load the A
        # (feedback) flop at a_block.
        u = UopConfig()
        u.enable_input(InpSel.CONST_0, 0)
        u.repeat_count = 1
        u.trigger = (Trigger.COUNT, Trigger.NONE, Trigger.NONE)
        u.next_uop = (1, 0, 0)
        for b in range(a_block + 1):
            u.datapath_config[b].pass_through_alu()
        u.datapath_config[a_block].alu_out_a_enable = ENABLE
        return u

    def _bubble():
        u = UopConfig()
        u.repeat_count = 1
        u.trigger = (Trigger.COUNT, Trigger.NONE, Trigger.NONE)
        u.next_uop = (2, 0, 0)
        return u

    def _work_regular():
        # 1x: m = f*A (blk0); h = m + g (blk1, A <- h). f=inp0, g=lane0.
        u = UopConfig()
        u.enable_input(InpSel.SRC_0, 0)
        u.enable_input(InpSel.SRC_1, 1)
        u.repeat_count = 1
        u.require_inp0 = ENABLE
        u.require_inp1 = ENABLE
        u.trigger = (Trigger.SRC_TENSOR_DONE, Trigger.COUNT, Trigger.NONE)
        u.next_uop = (0, 1, 0)
        dp = u.datapath_config
        dp[0].enable_alu(
            AluOp.MULTIPLY, AluInp.PREV_ALU_OUT, AluInp.NEXT_ALU_OUT_A)
        dp[0].pass_through_delay(0)
        dp[1].enable_alu(AluOp.ADD, AluInp.PREV_ALU_OUT, AluInp.PREV_DELAY_0)
        dp[1].alu_out_a_enable = ENABLE
        for b in range(2, 8):
            dp[b].pass_through_alu()
        u.enable_output(OutSel.ALU_OUT, OutPath.WR0_LO)
        return u

    def _work_pair():
        # 2X_1PORT: lanes 0<-f1(SRC_0_HI), 1<-g0(SRC_1), 2<-g1(SRC_1_HI),
        # 3<-f0(SRC_0); inp0 = f0 feeds blk0's ALU directly.
        u = UopConfig()
        u.enable_input(InpSel.SRC_0, 0)
        u.enable_input(InpSel.SRC_0_HI, 1)
        u.enable_input(InpSel.SRC_1, 2)
        u.enable_input(InpSel.SRC_1_HI, 3)
        u.enable_input(InpSel.SRC_0, 4)
        u.repeat_count = 1
        u.require_inp0 = ENABLE
        u.require_inp1 = ENABLE
        u.trigger = (Trigger.SRC_TENSOR_DONE, Trigger.COUNT, Trigger.NONE)
        u.next_uop = (0, 1, 0)
        dp = u.datapath_config
        # blk0: c = f0 * f1
        dp[0].enable_alu(
            AluOp.MULTIPLY, AluInp.PREV_ALU_OUT, AluInp.PREV_DELAY_0)
        dp[0].pass_through_delay(0, 1, 2, 3)
        # blk1: t = f1 * g0 ; lane4 <- c
        dp[1].enable_alu(
            AluOp.MULTIPLY, AluInp.PREV_DELAY_0, AluInp.PREV_DELAY_1)
        dp[1].pass_through_delay(1, 2, 3)
        dp[1].enable_delay_from_src(DelayInp.PREV_ALU_OUT, 4)
        # blk2: d = t + g1
        dp[2].enable_alu(AluOp.ADD, AluInp.PREV_ALU_OUT, AluInp.PREV_DELAY_2)
        dp[2].pass_through_delay(1, 3, 4)
        # blk3: m = c * A ; lane5 <- d ; lane0 <- A (capture h_prev)
        dp[3].enable_alu(
            AluOp.MULTIPLY, AluInp.PREV_DELAY_4, AluInp.NEXT_ALU_OUT_A)
        dp[3].pass_through_delay(1, 3)
        dp[3].enable_delay_from_src(DelayInp.PREV_ALU_OUT, 5)
        dp[3].enable_delay_from_src(DelayInp.NEXT_ALU_OUT_A, 0)
        # blk4: h2 = m + d ; A <- h2
        dp[4].enable_alu(AluOp.ADD, AluInp.PREV_ALU_OUT, AluInp.PREV_DELAY_5)
        dp[4].alu_out_a_enable = ENABLE
        dp[4].pass_through_delay(0, 1, 3)
        # blk5: m1 = f0 * h_prev ; lane2 <- h2
        dp[5].enable_alu(
            AluOp.MULTIPLY, AluInp.PREV_DELAY_3, AluInp.PREV_DELAY_0)
        dp[5].pass_through_delay(1)
        dp[5].enable_delay_from_src(DelayInp.PREV_ALU_OUT, 2)
        # blk6: h1 = m1 + g0
        dp[6].enable_alu(AluOp.ADD, AluInp.PREV_ALU_OUT, AluInp.PREV_DELAY_1)
        dp[6].pass_through_delay(2)
        # blk7: pass h1 on the ALU; keep h2 on lane2
        dp[7].pass_through_alu()
        dp[7].pass_through_delay(2)
        u.enable_output(OutSel.ALU_OUT, OutPath.WR0_LO)   # h1 (elem 2k)
        u.enable_output(OutSel.DELAY_2, OutPath.WR0_HI)   # h2 (elem 2k+1)
        return u

    spec = Spec(body=Src0 * C0 + Src1, reference=_pair_scan_ref)
    op = _RawDveOp(_PAIR_SCAN_NAME, spec, subdim=False, uops_sha={})
    D.OPS.append(op)
    D.CUSTOM_DVE_SPECS[op.name] = spec
    D._SUB_OPCODE_FOR_NAME[op.name] = D._CUSTOM_DVE_ROW_BASE + len(D.OPS) - 1
    opcode = D.get_dve_sub_opcode(op.name)
    for ver in ("v3", "v4"):
        sp = DveOpSpec(
            name=op.name,
            opcode=opcode,
            uops=[_seed(1), _bubble(), _work_regular()],
            uops_2x=[_seed(4), _bubble(), _work_pair()],
            perf_max=1,
            rd1_en=True,
        )
        sp.validate(ver)
        op.raw[ver] = sp
    return op


def _emit_pair_scan(nc, op, out, in0, in1, s0):
    inst = nc.vector._custom_dve(op, out=out, in0=in0, in1=in1, s0=s0)
    inst.ins.perf_max = 1  # engage the 2X_1PORT slot (byte-36[7:6])
    return inst


def build_minlstm_bass(repeat=1, use_fp8=None):
    if use_fp8 is None:
        use_fp8 = USE_FP8
    assert use_fp8, "v3 kernel is fp8-gate only"
    gate_fn_op = _register_gate_fn()
    pair_scan_op = _register_pair_scan()

    nc = bacc.Bacc("TRN2", debug=False, num_devices=B)
    f32 = mybir.dt.float32
    bf16 = mybir.dt.bfloat16
    fp8 = mybir.dt.float8e4

    xT = nc.dram_tensor("xt", [KI, 128, S], bf16, kind="ExternalInput").ap()
    xT8 = nc.dram_tensor(
        "xt8", [KI2, 128, 2, S], fp8, kind="ExternalInput").ap()
    wf8T = nc.dram_tensor(
        "wf8t", [KI2, 128, 2, H], fp8, kind="ExternalInput").ap()
    wi8T = nc.dram_tensor(
        "wi8t", [KI2, 128, 2, H], fp8, kind="ExternalInput").ap()
    whnT = nc.dram_tensor("whnt", [KI, 128, H], bf16, kind="ExternalInput").ap()
    bfb = nc.dram_tensor("bfb", [128, HB], f32, kind="ExternalInput").ap()
    bib = nc.dram_tensor("bib", [128, HB], f32, kind="ExternalInput").ap()
    bhnb = nc.dram_tensor("bhnb", [128, HB], f32, kind="ExternalInput").ap()
    h0b = nc.dram_tensor("h0b", [128, HB], f32, kind="ExternalInput").ap()
    outT = nc.dram_tensor(
        "outt", [HB, 128, S], bf16, kind="ExternalOutput").ap()

    Sig = mybir.ActivationFunctionType.Sigmoid
    Ident = mybir.ActivationFunctionType.Identity
    Alu = mybir.AluOpType

    with tile.TileContext(nc) as tc, nc.allow_low_precision(reason="bf16 gates"):
        with (
            tc.tile_pool(name="const", bufs=1) as const,
            tc.tile_pool(name="ps", bufs=2, space="PSUM") as ps,
            tc.tile_pool(name="p_sf", bufs=2) as p_sf,
            tc.tile_pool(name="p_si", bufs=2) as p_si,
            tc.tile_pool(name="p_htn", bufs=3) as p_htn,
            tc.tile_pool(name="p_inn", bufs=3) as p_inn,
            tc.tile_pool(name="p_fn", bufs=2) as p_fn,
            tc.tile_pool(name="p_g", bufs=2) as p_g,
            tc.tile_pool(name="hout", bufs=2) as hout,
        ):
            whn_sb = const.tile([128, KI, H], bf16, tag="whn")
            x_sb = const.tile([128, KI, S], bf16, tag="x")
            wf8_sb = const.tile([128, KI2, 2, H], fp8, tag="wf8")
            wi8_sb = const.tile([128, KI2, 2, H], fp8, tag="wi8")
            x8_sb = const.tile([128, KI2, 2, S], fp8, tag="x8")
            for ki2 in range(KI2):
                nc.sync.dma_start(
                    out=wf8_sb[:, ki2, :, :], in_=wf8T[ki2, :, :, :])
                nc.sync.dma_start(
                    out=wi8_sb[:, ki2, :, :], in_=wi8T[ki2, :, :, :])
            for ki in range(KI):
                nc.sync.dma_start(out=whn_sb[:, ki, :], in_=whnT[ki, :, :])
            for ch in range(NCH):
                for ki2 in range(KI2):
                    nc.sync.dma_start(
                        out=x8_sb[:, ki2, :, ts(ch, CH)],
                        in_=xT8[ki2, :, :, ts(ch, CH)])
                for ki in range(KI):
                    nc.sync.dma_start(
                        out=x_sb[:, ki, ts(ch, CH)], in_=xT[ki, :, ts(ch, CH)])
            bf_sb = const.tile([128, HB], f32, tag="bf")
            bi_sb = const.tile([128, HB], f32, tag="bi")
            bhn_sb = const.tile([128, HB], f32, tag="bhn")
            h0_sb = const.tile([128, HB], f32, tag="h0")
            nc.sync.dma_start(out=bf_sb, in_=bfb[:, :])
            nc.sync.dma_start(out=bi_sb, in_=bib[:, :])
            nc.sync.dma_start(out=bhn_sb, in_=bhnb[:, :])
            nc.sync.dma_start(out=h0_sb, in_=h0b[:, :])

            def gate_matmul(w_sb, hb, ch):
                pp = ps.tile([128, CH], f32, tag="pp")
                for ki in range(KI):
                    st, sp = (ki == 0), (ki == KI - 1)
                    for c in range(CH // MM):
                        nc.tensor.matmul(
                            pp[:, ts(c, MM)],
                            w_sb[:, ki, ds(hb * 128, 128)],
                            x_sb[:, ki, ds(ch * CH + c * MM, MM)],
                            start=st, stop=sp)
                return pp

            def gate_matmul_fp8(w8_sb, hb, ch):
                pp = ps.tile([128, CH], f32, tag="pp")
                for ki2 in range(KI2):
                    st, sp = (ki2 == 0), (ki2 == KI2 - 1)
                    for c in range(CH // MM):
                        nc.tensor.matmul(
                            pp[:, ts(c, MM)],
                            w8_sb[:, ki2, :, ds(hb * 128, 128)],
                            x8_sb[:, ki2, :, ds(ch * CH + c * MM, MM)],
                            start=st, stop=sp,
                            perf_mode=mybir.MatmulPerfMode.DoubleRow)
                return pp

            def emit_g(p, last):
                """Emit pending chunk p's g; close the row (scan+DMA) at
                ch==NCH-1. The final chunk uses one STT so its tail skips
                the GPSIMD hop."""
                hb, ch, fn_row, g_row, hh, htn, inn = p
                gsl = g_row[:, ts(ch, CH)]
                if last:
                    nc.vector.scalar_tensor_tensor(
                        gsl, fn_row[:, ts(ch, CH)], 1.0, htn,
                        Alu.subtract, Alu.mult)
                else:
                    nc.vector.tensor_tensor(gsl, inn, htn, Alu.mult)
                if ch == NCH - 1:
                    _emit_pair_scan(
                        nc, pair_scan_op, hh, fn_row, g_row,
                        h0_sb[:, hb : hb + 1])
                    eng = nc.gpsimd if (hb % 2) else nc.sync
                    eng.dma_start(out=outT[hb, :, :], in_=hh)

            def body(_i=None, pend=None, close=True):
                for hb in range(HB):
                    fn_row = p_fn.tile([128, S], bf16, tag="fn")
                    g_row = p_g.tile([128, S], bf16, tag="g")
                    hh = hout.tile([128, S], bf16, tag="hh")
                    for ch in range(NCH):
                        sc = 1.0 / FP8_SC
                        if pend is not None:
                            emit_g(pend, last=False)
                            pend = None
                        ppf = gate_matmul_fp8(wf8_sb, hb, ch)
                        sf = p_sf.tile([128, CH], f32, tag="sf")
                        nc.scalar.activation(
                            sf, ppf, Sig, bias=bf_sb[:, hb : hb + 1], scale=sc)
                        ppi = gate_matmul_fp8(wi8_sb, hb, ch)
                        si = p_si.tile([128, CH], f32, tag="si")
                        nc.scalar.activation(
                            si, ppi, Sig, bias=bi_sb[:, hb : hb + 1], scale=sc)
                        pph = gate_matmul(whn_sb, hb, ch)
                        htn = p_htn.tile([128, CH], bf16, tag="htn")
                        nc.scalar.activation(
                            htn, pph, Ident, bias=bhn_sb[:, hb : hb + 1])

                        nc.vector._custom_dve(
                            gate_fn_op, out=fn_row[:, ts(ch, CH)],
                            in0=sf, in1=si, s0=_RC0, s1=_RC1)
                        inn = p_inn.tile([128, CH], bf16, tag="inn")
                        nc.gpsimd.tensor_scalar(
                            inn, fn_row[:, ts(ch, CH)], 1.0, -1.0,
                            Alu.mult, Alu.add)
                        pend = (hb, ch, fn_row, g_row, hh, htn, inn)
                if close:
                    emit_g(pend, last=True)
                    return None
                return pend

            if repeat == 1:
                body()
            else:
                n_loop = repeat // UNROLL
                if n_loop > 0:
                    with tc.For_i(0, n_loop, 1) as _i:
                        p = None
                        for _u in range(UNROLL):
                            p = body(_i, pend=p, close=(_u == UNROLL - 1))
                for _u in range(repeat - max(n_loop, 0) * UNROLL):
                    body()
    nc.compile()
    return nc


def _dr8(W):
    """[H, In] -> DoubleRow fp8 layout [KI2, 128, 2, H]: plane pl of ki2
    holds k-tile (2*ki2+pl)."""
    return np.ascontiguousarray(
        W.T.reshape(KI2, 2, 128, H).transpose(0, 2, 1, 3).astype(FP8))


def _prep_core_inputs(x, h_0, W_f, b_f, W_i, b_i, W_h, b_h):
    """Build per-core input maps (host-side shard + layout transform)."""
    shared = {}
    shared["wf8t"] = _dr8(W_f * FP8_SC)
    shared["wi8t"] = _dr8(W_i * FP8_SC)
    shared["whnt"] = np.ascontiguousarray(
        (-W_h).T.reshape(KI, 128, H).astype(BF16))
    shared["bfb"] = np.ascontiguousarray(
        b_f.reshape(HB, 128).T.astype(np.float32))
    shared["bib"] = np.ascontiguousarray(
        b_i.reshape(HB, 128).T.astype(np.float32))
    shared["bhnb"] = np.ascontiguousarray(
        (-b_h).reshape(HB, 128).T.astype(np.float32))
    in_maps = []
    for b in range(B):
        m = dict(shared)
        m["xt"] = np.ascontiguousarray(x[b].T.reshape(KI, 128, S).astype(BF16))
        m["xt8"] = np.ascontiguousarray(
            x[b].T.reshape(KI2, 2, 128, S).transpose(0, 2, 1, 3).astype(FP8))
        m["h0b"] = np.ascontiguousarray(
            h_0[b].reshape(HB, 128).T.astype(np.float32))
        in_maps.append(m)
    return in_maps


def _run(in_maps, trace=False, repeat=1):
    key = f"nc{repeat}"
    if key not in _CACHE:
        _CACHE[key] = build_minlstm_bass(repeat=repeat)
    return run_bass_kernel_spmd(
        _CACHE[key], in_maps, core_ids=list(range(B)), trace=trace)


def kernel(x, h_0, W_f, b_f, W_i, b_i, W_h, b_h):
    x = np.asarray(x, dtype=np.float32)
    h_0 = np.asarray(h_0, dtype=np.float32)
    in_maps = _prep_core_inputs(
        x, h_0,
        np.asarray(W_f, np.float32), np.asarray(b_f, np.float32),
        np.asarray(W_i, np.float32), np.asarray(b_i, np.float32),
        np.asarray(W_h, np.float32), np.asarray(b_h, np.float32))
    res = _run(in_maps)
    out = np.empty((B, S, H), dtype=np.float32)
    for b in range(B):
        outt = res.results[b]["outt"]  # [HB, 128, S] bf16
        out[b] = outt.reshape(H, S).T.astype(np.float32)
    return out
